# revision 1
# baseline (speedup 1.0000x reference)
"""ContextualAttentionMask Trainium2 kernel.

Math (per batch sample):
  f: [256, 4096] feature map (channels x pixels), m: [4096] mask
  K[j, :]    = f[:, j] + 1e-7          (per-pixel 1x1 kernel)
  rstd[j]    = 1 / ||K[j, :]||_2
  raw[j, n]  = sum_c f[c, j] * f[c, n]          (only interior columns matter:
               the conv padding columns are dead compute - 1x1 kernels, the
               output at pad positions is cropped, softmax is per-column)
  att[j, n]  = softmax_j(rstd[j] * raw[j, n])
  fmap[c, n] = sum_j rstd[j] * m[j] * K[j, c] * att[j, n]
  final      = fmap * (1 - m) + f * m  ;  skip branch if mask nearly all-ones

Device computes (per core, unnormalized; host divides, blends, skip-branch):
  E[j, n] = exp(rstd[j] * raw[j, n] - 12)       (-12 keeps E in fp16 range;
                                                 cancels in the division)
  o[c, n] = sum_j km16[j, c] * E[j, n]     with km16 = fp16(rstd * m * K)
  s[n]    = sum_j E[j, n]

Sharding: 8 cores = 4 samples x 2 column-halves (2048 columns each).
Inputs are host-permuted so each core's own half is always columns 0..2047;
the j (softmax/contraction) order is irrelevant as long as f16/km16/rstd
agree. Tiny per-j scalars (rstd, rstd*m) and fp16 casts are host-side prep;
all heavy compute (2x 2048x4096x256 GEMMs + softmax) runs on device.
"""

import sys
from contextlib import ExitStack

import numpy as np

sys.path.insert(0, "/opt/trn_rl_repo")

from concourse import bacc, mybir, tile  # noqa: E402
from concourse.bass_utils import run_bass_kernel_spmd  # noqa: E402

FP32 = mybir.dt.float32
FP16 = mybir.dt.float16

CH = 256          # channels
J = 4096          # number of per-pixel kernels (= h*w)
NH = 2048         # columns handled per core (half of a sample)
EXP_BIAS = -12.0  # exp(x - 12) keeps values in fp16 range; cancels on host


def build_program(ch=CH, j_total=J, n_half=NH, bufs_sc=5, bufs_out=3,
                  bufs_e=13, loop_reps=1):
    """Emit the per-core Bass/Tile program (SPMD across 8 cores)."""
    assert ch % 128 == 0 and j_total % 128 == 0
    n_cb = ch // 128          # channel blocks
    n_jb = j_total // 128     # j blocks
    qs = min(512, n_half)     # output column chunk width
    nq = n_half // qs
    assert n_half % qs == 0

    nc = bacc.Bacc("TRN2", target_bir_lowering=False, debug=False, num_devices=8)

    f_d = nc.dram_tensor("f16", [ch, j_total], FP16, kind="ExternalInput").ap()
    km_d = nc.dram_tensor("km16", [j_total, ch], FP16, kind="ExternalInput").ap()
    rstd_d = nc.dram_tensor("rstd", [128, n_jb], FP32, kind="ExternalInput").ap()
    o_d = nc.dram_tensor("o", [ch, n_half], FP32, kind="ExternalOutput").ap()
    s_d = nc.dram_tensor("s", [1, n_half], FP32, kind="ExternalOutput").ap()

    with tile.TileContext(nc) as tc, ExitStack() as ctx:
        const_p = ctx.enter_context(tc.tile_pool(name="const", bufs=1))
        kt_p = ctx.enter_context(tc.tile_pool(name="kt", bufs=n_cb))
        km_p = ctx.enter_context(tc.tile_pool(name="km", bufs=n_jb))
        e_p = ctx.enter_context(tc.tile_pool(name="e", bufs=bufs_e))
        osb_p = ctx.enter_context(tc.tile_pool(name="osb", bufs=3))
        ssb_p = ctx.enter_context(tc.tile_pool(name="ssb", bufs=2))
        ps_sc = ctx.enter_context(
            tc.tile_pool(name="ps_sc", bufs=bufs_sc, space="PSUM"))
        ps_out = ctx.enter_context(
            tc.tile_pool(name="ps_out", bufs=bufs_out, space="PSUM"))

        ones32 = const_p.tile([128, 1], FP32)
        nc.vector.memset(ones32[:], 1.0)
        bias_e = const_p.tile([128, 1], FP32, tag="bias_e")
        nc.vector.memset(bias_e[:], EXP_BIAS)
        rstd = const_p.tile([128, n_jb], FP32, tag="rstd")

        # fp16 feature map, [c, j] layout; chunked DMA so matmuls start early.
        # The small rstd transfer rides after the first chunk pair: early
        # enough for the first exp, without delaying the first matmuls.
        kt = [
            kt_p.tile([128, j_total], FP16, tag="kt", name=f"kt{cb}")
            for cb in range(n_cb)
        ]
        if j_total >= 4096:  # small first chunks so the first matmuls start early
            bounds = [0, 512, 1024, 2048, j_total]
        else:
            bounds = list(range(0, j_total + 1, min(512, j_total)))
        for i, (q8, q9) in enumerate(zip(bounds[:-1], bounds[1:])):
            for cb in range(n_cb):
                nc.sync.dma_start(
                    out=kt[cb][:, q8:q9],
                    in_=f_d[cb * 128:(cb + 1) * 128, q8:q9],
                )
            if i == 0:
                nc.sync.dma_start(out=rstd[:], in_=rstd_d[:, :])

        # mask-and-norm-scaled kernels, [j, c] layout
        km = []
        for jb in range(n_jb):
            t = km_p.tile([128, ch], FP16, tag="km", name=f"km{jb}")
            nc.sync.dma_start(out=t[:], in_=km_d[jb * 128:(jb + 1) * 128, :])
            km.append(t)

        # fused main loop: scores -> exp -> sumexp & Km^T E accumulation.
        # The softmax denominator is folded partition-wise on the (idle) DVE
        # (acc[p, n] = sum_jb E[jb*128+p, n]); one fp32 ones-matmul per chunk
        # does the final 128-way fold, keeping the PE stream count minimal.
        # loop_reps > 1 repeats the identical work (timing experiments only).
        for q in [qq for _ in range(loop_reps) for qq in range(nq)]:
            nsl = slice(q * qs, (q + 1) * qs)
            sum_ps = ps_out.tile([1, qs], FP32, tag="out", name="sum_ps")
            acc = ssb_p.tile([128, qs], FP32, tag="acc", name="acc")
            out_ps = [
                ps_out.tile([128, qs], FP32, tag="out", name=f"out_ps{cb}")
                for cb in range(n_cb)
            ]
            # software pipeline: the exp-dependent matmuls trail the score
            # matmuls by D j-blocks, so the in-order PE queue never waits on
            # the ACT exp latency (recovers ~6 us of 117 ns/jb stalls).
            D = min(3, n_jb - 1)
            etiles = {}
            for jj in range(n_jb + D):
                if jj < n_jb:
                    jb = jj
                    jsl = slice(jb * 128, (jb + 1) * 128)
                    ps = ps_sc.tile([128, qs], FP32, tag="sc", name="ps")
                    for cb in range(n_cb):
                        nc.tensor.matmul(
                            ps[:], kt[cb][:, jsl], kt[cb][:, nsl],
                            start=(cb == 0), stop=(cb == n_cb - 1),
                        )
                    e = e_p.tile([128, qs], FP16, tag="e", name="e")
                    nc.scalar.activation(
                        e[:], ps[:], mybir.ActivationFunctionType.Exp,
                        bias=bias_e[:], scale=rstd[:, jb:jb + 1],
                    )
                    etiles[jb] = e
                if jj >= D:
                    jb = jj - D
                    e = etiles.pop(jb)
                    if jb == 0:
                        nc.vector.tensor_copy(acc[:], e[:])
                    else:
                        nc.vector.tensor_add(acc[:], acc[:], e[:])
                    for cb in range(n_cb):
                        nc.tensor.matmul(
                            out_ps[cb][:], km[jb][:, cb * 128:(cb + 1) * 128], e[:],
                            start=(jb == 0), stop=(jb == n_jb - 1),
                        )
            nc.tensor.matmul(sum_ps[:], ones32[:], acc[:], start=True, stop=True)
            srow = ssb_p.tile([1, qs], FP32, tag="srow", name="srow")
            nc.vector.tensor_copy(srow[:], sum_ps[:])
            nc.sync.dma_start(out=s_d[0:1, nsl], in_=srow[:])
            for cb in range(n_cb):
                osb = osb_p.tile([128, qs], FP32, tag="osb", name="osb")
                nc.vector.tensor_copy(osb[:], out_ps[cb][:])
                nc.sync.dma_start(out=o_d[cb * 128:(cb + 1) * 128, nsl], in_=osb[:])

    nc.compile()
    return nc


_CACHE = {}


def _get_program():
    if "nc" not in _CACHE:
        _CACHE["nc"] = build_program()
    return _CACHE["nc"]


def _get_runner():
    """Cached sharded executable over 8 cores (same program/plugin as
    run_bass_kernel_spmd's axon path, but without per-call retracing)."""
    if "runner" in _CACHE:
        return _CACHE["runner"]
    import jax
    from jax.sharding import Mesh, NamedSharding, PartitionSpec
    from jax.experimental.shard_map import shard_map
    from concourse import bass2jax, mybir
    from concourse.bass2jax import _bass_exec_p, partition_id_tensor

    nc = _get_program()
    bass2jax.install_neuronx_cc_hook()
    pname = nc.partition_id_tensor.name if nc.partition_id_tensor else None

    in_names, out_names, out_avals = [], [], []
    for alloc in nc.m.functions[0].allocations:
        if not isinstance(alloc, mybir.MemoryLocationSet):
            continue
        name = alloc.memorylocations[0].name
        if alloc.kind == "ExternalInput":
            if name != pname:
                in_names.append(name)
        elif alloc.kind == "ExternalOutput":
            out_names.append(name)
            out_avals.append(
                jax.core.ShapedArray(
                    tuple(alloc.tensor_shape), mybir.dt.np(alloc.dtype)
                )
            )
    n_params, n_outs = len(in_names), len(out_names)
    all_in = in_names + out_names + ([pname] if pname else [])

    def _body(*args):
        operands = list(args)
        if pname is not None:
            operands.append(partition_id_tensor())
        return tuple(_bass_exec_p.bind(
            *operands, out_avals=tuple(out_avals), in_names=tuple(all_in),
            out_names=tuple(out_names), lowering_input_output_aliases=(),
            sim_require_finite=True, sim_require_nnan=True, nc=nc,
        ))

    devices = jax.devices()[:8]
    mesh = Mesh(np.asarray(devices), ("core",))
    spec = NamedSharding(mesh, PartitionSpec("core"))
    fn = jax.jit(
        shard_map(
            _body, mesh=mesh,
            in_specs=(PartitionSpec("core"),) * (n_params + n_outs),
            out_specs=(PartitionSpec("core"),) * n_outs,
            check_rep=False,
        ),
        donate_argnums=tuple(range(n_params, n_params + n_outs)),
        keep_unused=True,
    )
    zero_host = [
        np.zeros((8 * a.shape[0], *a.shape[1:]), a.dtype) for a in out_avals
    ]

    def run(in_maps):
        concat_in = [
            np.concatenate([np.asarray(m[name]) for m in in_maps], axis=0)
            for name in in_names
        ]
        zeros = [jax.device_put(z, spec) for z in zero_host]
        out = fn(*concat_in, *zeros)
        return [
            {
                name: np.asarray(out[i]).reshape(8, *out_avals[i].shape)[c]
                for i, name in enumerate(out_names)
            }
            for c in range(8)
        ]

    _CACHE["runner"] = run
    return run


def make_in_maps(foreground, mask):
    """Per-core host-side input prep (permute so own half is first)."""
    bs, ch, h, w = foreground.shape
    hw = h * w
    half = hw // 2
    f = np.ascontiguousarray(foreground.reshape(bs, ch, hw), dtype=np.float32)
    m = np.ascontiguousarray(mask.reshape(bs, hw), dtype=np.float32)
    in_maps = []
    for b in range(bs):
        k = f[b] + np.float32(1e-7)                 # [ch, hw], reference's +1e-7
        rstd = 1.0 / np.sqrt((k * k).sum(axis=0, dtype=np.float64))  # [hw]
        rstd = rstd.astype(np.float32)
        f16 = f[b].astype(np.float16)               # [ch, hw]
        km16 = ((rstd * m[b])[:, None] * k.T).astype(np.float16)  # [hw, ch]
        for hh in range(2):
            if hh == 0:
                fc, kmc, rc = f16, km16, rstd
            else:  # swap the two column-halves so own half comes first
                fc = np.concatenate([f16[:, half:], f16[:, :half]], axis=1)
                kmc = np.concatenate([km16[half:], km16[:half]], axis=0)
                rc = np.concatenate([rstd[half:], rstd[:half]])
            in_maps.append({
                "f16": np.ascontiguousarray(fc),
                "km16": np.ascontiguousarray(kmc),
                "rstd": np.ascontiguousarray(rc.reshape(hw // 128, 128).T),
            })
    return in_maps


def kernel(foreground, mask):
    foreground = np.asarray(foreground, dtype=np.float32)
    mask = np.asarray(mask, dtype=np.float32)
    bs, ch, h, w = foreground.shape
    hw = h * w

    in_maps = make_in_maps(foreground, mask)
    try:
        results = _get_runner()(in_maps)
    except Exception:
        # robust fallback: the generic SPMD entry point
        res = run_bass_kernel_spmd(_get_program(), in_maps, list(range(8)))
        results = res.results

    fmap = np.empty((bs, ch, h, w), dtype=np.float32)
    rows = h // 2
    for core in range(8):
        b, hh = core // 2, core % 2
        o = results[core]["o"]       # [ch, hw/2] unnormalized
        s = results[core]["s"]       # [1, hw/2] softmax denominator
        fmap[b, :, hh * rows:(hh + 1) * rows, :] = (o / s).reshape(ch, rows, w)

    mm = mask[:, 0:1]                    # [bs, 1, h, w]
    final = fmap * (1.0 - mm) + foreground * mm
    skip = mask.sum(axis=(1, 2, 3)) > (hw - 10)
    final[skip] = foreground[skip]
    return final.astype(np.float32)



# revision 2
# speedup vs baseline: 1.1243x; 1.1243x over previous
"""ContextualAttentionMask Trainium2 kernel (fp8 DoubleRow version).

Math (per batch sample):
  f: [256, 4096] feature map (channels x pixels), m: [4096] mask
  K[j, :]    = f[:, j] + 1e-7          (per-pixel 1x1 kernel)
  rstd[j]    = 1 / ||K[j, :]||_2
  raw[j, n]  = rstd[j] * sum_c K[c, j] * f[c, n]
  att[j, n]  = softmax_j(raw[j, n])
  fmap[c, n] = sum_j rstd[j] * m[j] * K[j, c] * att[j, n]
  final      = fmap * (1 - m) + f * m  ;  skip branch if mask nearly all-ones

Device computes (per core, unnormalized; host divides, blends, skip-branch):
  E[j, n] = exp(raw[j, n] - 9)   as fp8e5  (bias keeps E in e5m2 range and
                                  ~all of the softmax tail above the e5m2
                                  subnormal floor; cancels in the division)
  o[c, n] = sum_j km8[j, c] * E[j, n]      (km8 = e4m3(rstd * m * K))
  s[n]    = sum_j E[j, n]                  (ones-matmul on PE)

All three matmul families run fp8 with DoubleRow (2 contraction rows per
partition): scores contract ch=256 as 128x2, output/sum contract j in
pairs of 128-blocks. rstd is folded into the scores lhsT on the host so
the exp needs only a constant bias -> ACT instructions can span j-block
pairs ([128, 1024]) without per-row scale vectors.

Sharding: 8 cores = 4 samples x 2 column-halves (2048 columns each).
"""

import sys
from contextlib import ExitStack

import numpy as np

sys.path.insert(0, "/opt/trn_rl_repo")

import ml_dtypes  # noqa: E402

from concourse import bacc, mybir, tile  # noqa: E402
from concourse.bass_utils import run_bass_kernel_spmd  # noqa: E402

FP32 = mybir.dt.float32
FP8E4 = mybir.dt.float8e4
FP8E5 = mybir.dt.float8e5
E4 = ml_dtypes.float8_e4m3
E5 = ml_dtypes.float8_e5m2

CH = 256          # channels
J = 4096          # number of per-pixel kernels (= h*w)
NH = 2048         # columns handled per core (half of a sample)
EXP_BIAS = -9.0   # exp(x - 9) keeps E in fp8e5 range; cancels on host
DR = mybir.MatmulPerfMode.DoubleRow


def build_program(ch=CH, j_total=J, n_half=NH, loop_reps=1, bufs_e=5,
                  depth=2):
    """Emit the per-core Bass/Tile program (SPMD across 8 cores)."""
    n_pair = j_total // 256   # j-block pairs (DoubleRow granularity)
    qs = 512                  # output column chunk width (one PSUM bank)
    nq = n_half // qs

    nc = bacc.Bacc("TRN2", target_bir_lowering=False, debug=False, num_devices=8)

    fs_d = nc.dram_tensor("fs8", [128, 2, j_total], FP8E4, kind="ExternalInput").ap()
    f_d = nc.dram_tensor("f8", [128, 2, n_half], FP8E4, kind="ExternalInput").ap()
    km_d = nc.dram_tensor("km8", [128, n_pair, 2, ch], FP8E4,
                          kind="ExternalInput").ap()
    o_d = nc.dram_tensor("o", [ch, n_half], FP32, kind="ExternalOutput").ap()
    s_d = nc.dram_tensor("s", [1, n_half], FP32, kind="ExternalOutput").ap()

    with tile.TileContext(nc) as tc, ExitStack() as ctx:
        const_p = ctx.enter_context(tc.tile_pool(name="const", bufs=1))
        fs_p = ctx.enter_context(tc.tile_pool(name="fs", bufs=1))
        f8_p = ctx.enter_context(tc.tile_pool(name="f8", bufs=1))
        km_p = ctx.enter_context(tc.tile_pool(name="km", bufs=1))
        e_p = ctx.enter_context(tc.tile_pool(name="e", bufs=bufs_e))
        osb_p = ctx.enter_context(tc.tile_pool(name="osb", bufs=3))
        ssb_p = ctx.enter_context(tc.tile_pool(name="ssb", bufs=2))
        ps_sc = ctx.enter_context(
            tc.tile_pool(name="ps_sc", bufs=2, space="PSUM"))
        ps_out = ctx.enter_context(
            tc.tile_pool(name="ps_out", bufs=4, space="PSUM"))

        # DoubleRow ldweights needs pair-dim byte-stride % 16 == 0
        ones8 = const_p.tile([128, 2, 16], FP8E4, tag="ones")
        nc.vector.memset(ones8[:], 1.0)
        bias_e = const_p.tile([128, 1], FP32, tag="bias_e")
        nc.vector.memset(bias_e[:], EXP_BIAS)

        fs = fs_p.tile([128, 2, j_total], FP8E4, tag="fs")
        f8 = f8_p.tile([128, 2, n_half], FP8E4, tag="f8")
        km = km_p.tile([128, n_pair, 2, ch], FP8E4, tag="km")

        # chunked input DMA so the first matmuls start early
        nc.sync.dma_start(out=f8[:, :, 0:qs], in_=f_d[:, :, 0:qs])
        bounds = [0, 512, 1024, 2048, j_total]
        for q8, q9 in zip(bounds[:-1], bounds[1:]):
            nc.sync.dma_start(out=fs[:, :, q8:q9], in_=fs_d[:, :, q8:q9])
        nc.sync.dma_start(out=km[:, 0:4], in_=km_d[:, 0:4])
        nc.sync.dma_start(out=km[:, 4:n_pair], in_=km_d[:, 4:n_pair])
        for q in range(1, nq):
            nc.sync.dma_start(out=f8[:, :, q * qs:(q + 1) * qs],
                              in_=f_d[:, :, q * qs:(q + 1) * qs])

        # fused main loop: scores -> exp -> {Km^T E, 1^T E} accumulation.
        # Software pipeline: exp-dependent matmuls trail the score matmuls
        # by `depth` pairs so the in-order PE queue never waits on ACT.
        for q in [qq for _ in range(loop_reps) for qq in range(nq)]:
            nsl = slice(q * qs, (q + 1) * qs)
            out_ps = [
                ps_out.tile([128, qs], FP32, tag="out", name=f"out_ps{cb}")
                for cb in range(2)
            ]
            sum_ps = ps_out.tile([1, qs], FP32, tag="out", name="sum_ps")
            etiles = {}
            for pp in range(n_pair + depth):
                if pp < n_pair:
                    ps = ps_sc.tile([128, 2, qs], FP32, tag="sc", name="ps")
                    for i in range(2):
                        jb = pp * 2 + i
                        nc.tensor.matmul(
                            ps[:, i, :],
                            fs[:, :, jb * 128:(jb + 1) * 128],
                            f8[:, :, nsl],
                            start=True, stop=True, perf_mode=DR,
                        )
                    e = e_p.tile([128, 2, qs], FP8E5, tag="e", name="e")
                    nc.scalar.activation(
                        e[:], ps[:], mybir.ActivationFunctionType.Exp,
                        bias=bias_e[:],
                    )
                    etiles[pp] = e
                if pp >= depth:
                    p0 = pp - depth
                    e = etiles.pop(p0)
                    for cb in range(2):
                        nc.tensor.matmul(
                            out_ps[cb][:],
                            km[:, p0, :, cb * 128:(cb + 1) * 128],
                            e[:],
                            start=(p0 == 0), stop=(p0 == n_pair - 1),
                            perf_mode=DR,
                        )
                    nc.tensor.matmul(
                        sum_ps[:], ones8[:, :, 0:1], e[:],
                        start=(p0 == 0), stop=(p0 == n_pair - 1),
                        perf_mode=DR,
                    )
            srow = ssb_p.tile([1, qs], FP32, tag="srow", name="srow")
            nc.vector.tensor_copy(srow[:], sum_ps[:])
            nc.sync.dma_start(out=s_d[0:1, nsl], in_=srow[:])
            for cb in range(2):
                osb = osb_p.tile([128, qs], FP32, tag="osb", name="osb")
                nc.vector.tensor_copy(osb[:], out_ps[cb][:])
                nc.sync.dma_start(out=o_d[cb * 128:(cb + 1) * 128, nsl], in_=osb[:])

    nc.compile()
    return nc


_CACHE = {}


def _get_program():
    if "nc" not in _CACHE:
        _CACHE["nc"] = build_program()
    return _CACHE["nc"]


def _get_runner():
    """Cached sharded executable over 8 cores (same program/plugin as
    run_bass_kernel_spmd's axon path, but without per-call retracing)."""
    if "runner" in _CACHE:
        return _CACHE["runner"]
    import jax
    from jax.sharding import Mesh, NamedSharding, PartitionSpec
    from jax.experimental.shard_map import shard_map
    from concourse import bass2jax, mybir
    from concourse.bass2jax import _bass_exec_p, partition_id_tensor

    nc = _get_program()
    bass2jax.install_neuronx_cc_hook()
    pname = nc.partition_id_tensor.name if nc.partition_id_tensor else None

    in_names, out_names, out_avals = [], [], []
    for alloc in nc.m.functions[0].allocations:
        if not isinstance(alloc, mybir.MemoryLocationSet):
            continue
        name = alloc.memorylocations[0].name
        if alloc.kind == "ExternalInput":
            if name != pname:
                in_names.append(name)
        elif alloc.kind == "ExternalOutput":
            out_names.append(name)
            out_avals.append(
                jax.core.ShapedArray(
                    tuple(alloc.tensor_shape), mybir.dt.np(alloc.dtype)
                )
            )
    n_params, n_outs = len(in_names), len(out_names)
    all_in = in_names + out_names + ([pname] if pname else [])

    def _body(*args):
        operands = list(args)
        if pname is not None:
            operands.append(partition_id_tensor())
        return tuple(_bass_exec_p.bind(
            *operands, out_avals=tuple(out_avals), in_names=tuple(all_in),
            out_names=tuple(out_names), lowering_input_output_aliases=(),
            sim_require_finite=True, sim_require_nnan=True, nc=nc,
        ))

    devices = jax.devices()[:8]
    mesh = Mesh(np.asarray(devices), ("core",))
    spec = NamedSharding(mesh, PartitionSpec("core"))
    fn = jax.jit(
        shard_map(
            _body, mesh=mesh,
            in_specs=(PartitionSpec("core"),) * (n_params + n_outs),
            out_specs=(PartitionSpec("core"),) * n_outs,
            check_rep=False,
        ),
        donate_argnums=tuple(range(n_params, n_params + n_outs)),
        keep_unused=True,
    )
    zero_host = [
        np.zeros((8 * a.shape[0], *a.shape[1:]), a.dtype) for a in out_avals
    ]

    def run(in_maps):
        concat_in = [
            np.concatenate([np.asarray(m[name]) for m in in_maps], axis=0)
            for name in in_names
        ]
        zeros = [jax.device_put(z, spec) for z in zero_host]
        out = fn(*concat_in, *zeros)
        return [
            {
                name: np.asarray(out[i]).reshape(8, *out_avals[i].shape)[c]
                for i, name in enumerate(out_names)
            }
            for c in range(8)
        ]

    _CACHE["runner"] = run
    return run


def make_in_maps(foreground, mask):
    """Per-core host-side input prep: fp8 casts + DoubleRow layouts."""
    bs, ch, h, w = foreground.shape
    hw = h * w
    half = hw // 2
    f = np.ascontiguousarray(foreground.reshape(bs, ch, hw), dtype=np.float32)
    m = np.ascontiguousarray(mask.reshape(bs, hw), dtype=np.float32)
    in_maps = []
    for b in range(bs):
        k = f[b] + np.float32(1e-7)                 # [ch, hw], reference's +1e-7
        rstd = 1.0 / np.sqrt((k * k).sum(axis=0, dtype=np.float64))  # [hw]
        rstd = rstd.astype(np.float32)
        # scores lhsT: rstd folded in; [128, 2, hw] ch-pair layout
        fs8 = (rstd[None, :] * k).reshape(2, 128, hw).transpose(1, 0, 2)
        fs8 = np.ascontiguousarray(fs8).astype(E4)
        # out lhsT: km[j, c] = rstd_j m_j K[j, c]; [128, pairs, 2, ch]
        km_full = ((rstd * m[b])[:, None] * k.T)     # [hw, ch]
        km8 = km_full.reshape(hw // 256, 2, 128, ch).transpose(2, 0, 1, 3)
        km8 = np.ascontiguousarray(km8).astype(E4)
        for hh in range(2):
            fh = f[b][:, hh * half:(hh + 1) * half]  # [ch, half]
            f8 = fh.reshape(2, 128, half).transpose(1, 0, 2)
            in_maps.append({
                "fs8": fs8,
                "f8": np.ascontiguousarray(f8).astype(E4),
                "km8": km8,
            })
    return in_maps


def kernel(foreground, mask):
    foreground = np.asarray(foreground, dtype=np.float32)
    mask = np.asarray(mask, dtype=np.float32)
    bs, ch, h, w = foreground.shape
    hw = h * w

    in_maps = make_in_maps(foreground, mask)
    try:
        results = _get_runner()(in_maps)
    except Exception:
        # robust fallback: the generic SPMD entry point
        res = run_bass_kernel_spmd(_get_program(), in_maps, list(range(8)))
        results = res.results

    fmap = np.empty((bs, ch, h, w), dtype=np.float32)
    rows = h // 2
    for core in range(8):
        b, hh = core // 2, core % 2
        o = results[core]["o"]       # [ch, hw/2] unnormalized
        s = results[core]["s"]       # [1, hw/2] softmax denominator
        fmap[b, :, hh * rows:(hh + 1) * rows, :] = (o / s).reshape(ch, rows, w)

    mm = mask[:, 0:1]                    # [bs, 1, h, w]
    final = fmap * (1.0 - mm) + foreground * mm
    skip = mask.sum(axis=(1, 2, 3)) > (hw - 10)
    final[skip] = foreground[skip]
    return final.astype(np.float32)


# revision 9
# speedup vs baseline: 22.8680x; 20.3399x over previous
"""ContextualAttentionMask Trainium2 kernel (fp8 DoubleRow version).

Math (per batch sample):
  f: [256, 4096] feature map (channels x pixels), m: [4096] mask
  K[j, :]    = f[:, j] + 1e-7          (per-pixel 1x1 kernel)
  rstd[j]    = 1 / ||K[j, :]||_2
  raw[j, n]  = rstd[j] * sum_c K[c, j] * f[c, n]
  att[j, n]  = softmax_j(raw[j, n])
  fmap[c, n] = sum_j rstd[j] * m[j] * K[j, c] * att[j, n]
  final      = fmap * (1 - m) + f * m  ;  skip branch if mask nearly all-ones

Device computes (per core, unnormalized; host divides, blends, skip-branch):
  E[j, n] = exp(raw[j, n] - 9)   as fp8e5  (bias keeps E in e5m2 range and
                                  ~all of the softmax tail above the e5m2
                                  subnormal floor; cancels in the division)
  o[c, n] = sum_j km8[j, c] * E[j, n]      (km8 = e4m3(rstd * m * K))
  s[n]    = sum_j E[j, n]                  (ones-matmul on PE)

All three matmul families run fp8 with DoubleRow (2 contraction rows per
partition): scores contract ch=256 as 128x2, output/sum contract j in
pairs of 128-blocks. rstd is folded into the scores lhsT on the host so
the exp needs only a constant bias -> ACT instructions can span j-block
pairs ([128, 1024]) without per-row scale vectors.

Sharding: 8 cores = 4 samples x 2 column-halves (2048 columns each).
"""

import sys
from contextlib import ExitStack

import numpy as np

sys.path.insert(0, "/opt/trn_rl_repo")

import ml_dtypes  # noqa: E402

from concourse import bacc, mybir, tile  # noqa: E402
from concourse.bass_utils import run_bass_kernel_spmd  # noqa: E402

FP32 = mybir.dt.float32
FP16 = mybir.dt.float16
FP8E4 = mybir.dt.float8e4
FP8E5 = mybir.dt.float8e5
E4 = ml_dtypes.float8_e4m3
E5 = ml_dtypes.float8_e5m2

CH = 256          # channels
J = 4096          # number of per-pixel kernels (= h*w)
NH = 2048         # columns handled per core (half of a sample)
EXP_BIAS = -9.0   # exp(x - 9) keeps E in fp8e5 range; cancels on host
DR = mybir.MatmulPerfMode.DoubleRow


def build_program(ch=CH, j_total=J, n_half=NH, loop_reps=1, bufs_e=5,
                  depth=2):
    """Emit the per-core Bass/Tile program (SPMD across 8 cores)."""
    n_pair = j_total // 256   # j-block pairs (DoubleRow granularity)
    qs = 512                  # output column chunk width (one PSUM bank)
    nq = n_half // qs

    nc = bacc.Bacc("TRN2", target_bir_lowering=False, debug=False, num_devices=8)

    fs_d = nc.dram_tensor("fs8", [128, 2, j_total], FP8E4, kind="ExternalInput").ap()
    f_d = nc.dram_tensor("f8", [128, 2, n_half], FP8E4, kind="ExternalInput").ap()
    km_d = nc.dram_tensor("km8", [128, n_pair, 2, ch], FP8E4,
                          kind="ExternalInput").ap()
    o_d = nc.dram_tensor("o", [ch, n_half], FP16, kind="ExternalOutput").ap()
    s_d = nc.dram_tensor("s", [1, n_half], FP32, kind="ExternalOutput").ap()

    with tile.TileContext(nc) as tc, ExitStack() as ctx:
        const_p = ctx.enter_context(tc.tile_pool(name="const", bufs=1))
        fs_p = ctx.enter_context(tc.tile_pool(name="fs", bufs=1))
        f8_p = ctx.enter_context(tc.tile_pool(name="f8", bufs=1))
        km_p = ctx.enter_context(tc.tile_pool(name="km", bufs=1))
        e_p = ctx.enter_context(tc.tile_pool(name="e", bufs=bufs_e))
        osb_p = ctx.enter_context(tc.tile_pool(name="osb", bufs=3))
        ssb_p = ctx.enter_context(tc.tile_pool(name="ssb", bufs=2))
        ps_sc = ctx.enter_context(
            tc.tile_pool(name="ps_sc", bufs=2, space="PSUM"))
        ps_out = ctx.enter_context(
            tc.tile_pool(name="ps_out", bufs=4, space="PSUM"))

        # DoubleRow ldweights needs pair-dim byte-stride % 16 == 0
        ones8 = const_p.tile([128, 2, 16], FP8E4, tag="ones")
        nc.vector.memset(ones8[:], 1.0)
        bias_e = const_p.tile([128, 1], FP32, tag="bias_e")
        nc.vector.memset(bias_e[:], EXP_BIAS)
        # dummy exp: pulls the ACT exp-table load into the DMA window
        warm = const_p.tile([128, 1], FP32, tag="warm")
        nc.scalar.activation(warm[:], bias_e[:],
                             mybir.ActivationFunctionType.Exp)

        fs = fs_p.tile([128, 2, j_total], FP8E4, tag="fs")
        f8 = f8_p.tile([128, 2, n_half], FP8E4, tag="f8")
        km = km_p.tile([128, n_pair, 2, ch], FP8E4, tag="km")

        # input DMA ordered by first-use time; the first critical transfers
        # go out on separate engine queues so their DGE setups overlap
        nc.sync.dma_start(out=f8[:, :, 0:qs], in_=f_d[:, :, 0:qs])
        nc.gpsimd.dma_start(out=fs[:, :, 0:512], in_=fs_d[:, :, 0:512])
        nc.scalar.dma_start(out=km[:, 0:2], in_=km_d[:, 0:2])
        nc.sync.dma_start(out=fs[:, :, 512:1024], in_=fs_d[:, :, 512:1024])
        nc.gpsimd.dma_start(out=km[:, 2:6], in_=km_d[:, 2:6])
        nc.sync.dma_start(out=fs[:, :, 1024:2048], in_=fs_d[:, :, 1024:2048])
        nc.gpsimd.dma_start(out=fs[:, :, 2048:j_total],
                            in_=fs_d[:, :, 2048:j_total])
        nc.gpsimd.dma_start(out=km[:, 6:n_pair], in_=km_d[:, 6:n_pair])
        for q in range(1, nq):
            nc.sync.dma_start(out=f8[:, :, q * qs:(q + 1) * qs],
                              in_=f_d[:, :, q * qs:(q + 1) * qs])

        # fused main loop: scores -> exp -> {Km^T E, 1^T E} accumulation.
        # Software pipeline: exp-dependent matmuls trail the score matmuls
        # by `depth` pairs so the in-order PE queue never waits on ACT.
        for q in [qq for _ in range(loop_reps) for qq in range(nq)]:
            nsl = slice(q * qs, (q + 1) * qs)
            out_ps = [
                ps_out.tile([128, qs], FP32, tag="out", name=f"out_ps{cb}")
                for cb in range(2)
            ]
            sum_ps = ps_out.tile([1, qs], FP32, tag="out", name="sum_ps")
            etiles = {}
            for pp in range(n_pair + depth):
                if pp < n_pair:
                    ps = ps_sc.tile([128, 2, qs], FP32, tag="sc", name="ps")
                    for i in range(2):
                        jb = pp * 2 + i
                        nc.tensor.matmul(
                            ps[:, i, :],
                            fs[:, :, jb * 128:(jb + 1) * 128],
                            f8[:, :, nsl],
                            start=True, stop=True, perf_mode=DR,
                        )
                    e = e_p.tile([128, 2, qs], FP8E5, tag="e", name="e")
                    nc.scalar.activation(
                        e[:], ps[:], mybir.ActivationFunctionType.Exp,
                        bias=bias_e[:],
                    )
                    etiles[pp] = e
                if pp >= depth:
                    p0 = pp - depth
                    e = etiles.pop(p0)
                    for cb in range(2):
                        nc.tensor.matmul(
                            out_ps[cb][:],
                            km[:, p0, :, cb * 128:(cb + 1) * 128],
                            e[:],
                            start=(p0 == 0), stop=(p0 == n_pair - 1),
                            perf_mode=DR,
                        )
                    nc.tensor.matmul(
                        sum_ps[:], ones8[:, :, 0:1], e[:],
                        start=(p0 == 0), stop=(p0 == n_pair - 1),
                        perf_mode=DR,
                    )
            last = q == nq - 1
            srow = ssb_p.tile([1, qs], FP32, tag="srow", name="srow")
            nc.vector.tensor_copy(srow[:], sum_ps[:])
            (nc.gpsimd if last else nc.sync).dma_start(
                out=s_d[0:1, nsl], in_=srow[:])
            for cb in range(2):
                osb = osb_p.tile([128, qs], FP16, tag="osb", name="osb")
                if last and cb == 0:
                    # ACT is idle after its final exp; splitting the drain
                    # copies between ACT and DVE shortens the program tail
                    nc.scalar.copy(osb[:], out_ps[cb][:])
                else:
                    nc.vector.tensor_copy(osb[:], out_ps[cb][:])
                eng = (nc.scalar if cb == 0 else nc.sync) if last else nc.sync
                eng.dma_start(out=o_d[cb * 128:(cb + 1) * 128, nsl], in_=osb[:])

    nc.compile()
    return nc


_CACHE = {}


def _get_program():
    if "nc" not in _CACHE:
        _CACHE["nc"] = build_program()
    return _CACHE["nc"]


def _get_runner():
    """Cached sharded executable over 8 cores (same program/plugin as
    run_bass_kernel_spmd's axon path, but without per-call retracing)."""
    if "runner" in _CACHE:
        return _CACHE["runner"]
    import jax
    from jax.sharding import Mesh, NamedSharding, PartitionSpec
    from jax.experimental.shard_map import shard_map
    from concourse import bass2jax, mybir
    from concourse.bass2jax import _bass_exec_p, partition_id_tensor

    nc = _get_program()
    bass2jax.install_neuronx_cc_hook()
    pname = nc.partition_id_tensor.name if nc.partition_id_tensor else None

    in_names, out_names, out_avals = [], [], []
    for alloc in nc.m.functions[0].allocations:
        if not isinstance(alloc, mybir.MemoryLocationSet):
            continue
        name = alloc.memorylocations[0].name
        if alloc.kind == "ExternalInput":
            if name != pname:
                in_names.append(name)
        elif alloc.kind == "ExternalOutput":
            out_names.append(name)
            out_avals.append(
                jax.core.ShapedArray(
                    tuple(alloc.tensor_shape), mybir.dt.np(alloc.dtype)
                )
            )
    n_params, n_outs = len(in_names), len(out_names)
    all_in = in_names + out_names + ([pname] if pname else [])

    def _body(*args):
        operands = list(args)
        if pname is not None:
            operands.append(partition_id_tensor())
        return tuple(_bass_exec_p.bind(
            *operands, out_avals=tuple(out_avals), in_names=tuple(all_in),
            out_names=tuple(out_names), lowering_input_output_aliases=(),
            sim_require_finite=True, sim_require_nnan=True, nc=nc,
        ))

    devices = jax.devices()[:8]
    mesh = Mesh(np.asarray(devices), ("core",))
    spec = NamedSharding(mesh, PartitionSpec("core"))
    fn = jax.jit(
        shard_map(
            _body, mesh=mesh,
            in_specs=(PartitionSpec("core"),) * (n_params + n_outs),
            out_specs=(PartitionSpec("core"),) * n_outs,
            check_rep=False,
        ),
        donate_argnums=tuple(range(n_params, n_params + n_outs)),
        keep_unused=True,
    )
    zero_host = [
        np.zeros((8 * a.shape[0], *a.shape[1:]), a.dtype) for a in out_avals
    ]

    def run(in_maps):
        concat_in = [
            np.concatenate([np.asarray(m[name]) for m in in_maps], axis=0)
            for name in in_names
        ]
        zeros = [jax.device_put(z, spec) for z in zero_host]
        out = fn(*concat_in, *zeros)
        return [
            {
                name: np.asarray(out[i]).reshape(8, *out_avals[i].shape)[c]
                for i, name in enumerate(out_names)
            }
            for c in range(8)
        ]

    _CACHE["runner"] = run
    return run


def make_in_maps(foreground, mask):
    """Per-core host-side input prep: fp8 casts + DoubleRow layouts."""
    bs, ch, h, w = foreground.shape
    hw = h * w
    half = hw // 2
    f = np.ascontiguousarray(foreground.reshape(bs, ch, hw), dtype=np.float32)
    m = np.ascontiguousarray(mask.reshape(bs, hw), dtype=np.float32)
    in_maps = []
    for b in range(bs):
        k = f[b] + np.float32(1e-7)                 # [ch, hw], reference's +1e-7
        rstd = 1.0 / np.sqrt((k * k).sum(axis=0, dtype=np.float64))  # [hw]
        rstd = rstd.astype(np.float32)
        # scores lhsT: rstd folded in; [128, 2, hw] ch-pair layout
        fs8 = (rstd[None, :] * k).reshape(2, 128, hw).transpose(1, 0, 2)
        fs8 = np.ascontiguousarray(fs8).astype(E4)
        # out lhsT: km[j, c] = rstd_j m_j K[j, c]; [128, pairs, 2, ch]
        km_full = ((rstd * m[b])[:, None] * k.T)     # [hw, ch]
        km8 = km_full.reshape(hw // 256, 2, 128, ch).transpose(2, 0, 1, 3)
        km8 = np.ascontiguousarray(km8).astype(E4)
        for hh in range(2):
            fh = f[b][:, hh * half:(hh + 1) * half]  # [ch, half]
            f8 = fh.reshape(2, 128, half).transpose(1, 0, 2)
            in_maps.append({
                "fs8": fs8,
                "f8": np.ascontiguousarray(f8).astype(E4),
                "km8": km8,
            })
    return in_maps


def kernel(foreground, mask):
    foreground = np.asarray(foreground, dtype=np.float32)
    mask = np.asarray(mask, dtype=np.float32)
    bs, ch, h, w = foreground.shape
    hw = h * w

    in_maps = make_in_maps(foreground, mask)
    try:
        results = _get_runner()(in_maps)
    except Exception:
        # robust fallback: the generic SPMD entry point
        res = run_bass_kernel_spmd(_get_program(), in_maps, list(range(8)))
        results = res.results

    fmap = np.empty((bs, ch, h, w), dtype=np.float32)
    rows = h // 2
    for core in range(8):
        b, hh = core // 2, core % 2
        o = results[core]["o"]       # [ch, hw/2] unnormalized
        s = results[core]["s"]       # [1, hw/2] softmax denominator
        fmap[b, :, hh * rows:(hh + 1) * rows, :] = (o / s).reshape(ch, rows, w)

    mm = mask[:, 0:1]                    # [bs, 1, h, w]
    final = fmap * (1.0 - mm) + foreground * mm
    skip = mask.sum(axis=(1, 2, 3)) > (hw - 10)
    final[skip] = foreground[skip]
    return final.astype(np.float32)
